# revision 21
# baseline (speedup 1.0000x reference)
"""Self-contained Trainium2 (Bass/Tile) DeformConv2d kernel.

kernel(x, offset, weight) -> np.ndarray [B, Cout, H, W] float32.
Data-parallel over batch: one SPMD Bass program per NeuronCore (8 cores).

v2 design (vs v1 baseline):
- All bilinear weights / gather indices / gather table are built on the host
  (numpy) and passed as DRAM inputs; no on-device prep phases.
- Gather table is a bf16 "pair-row" layout: entry (y, x) holds image rows y
  and y+1 at column x concatenated (2C values). One 4C-value descriptor per
  sample covers all 4 bilinear neighbors (half the gather instructions and
  descriptor-gen of v1; DMA bytes unchanged).
- DVE does only the 4 per-sample weight multiplies (tensor_scalar, 4x mode).
- PE transpose-accumulates the 4 pieces into channel-major val (PSUM), then
  runs the per-tap GEMM.
- All PSUM->SBUF copies run on the otherwise-idle Activation engine.
- Output is written bf16 and upcast on the host.
"""
import sys
import numpy as np
import ml_dtypes

for _p in ("/opt/trn_rl_repo",):
    if _p not in sys.path:
        sys.path.insert(0, _p)

import concourse.bass as bass
import concourse.mybir as mybir
import concourse.tile as tile
from concourse import bacc
from concourse.masks import make_identity
from concourse.bass_utils import run_bass_kernel_spmd

f32 = mybir.dt.float32
bf16 = mybir.dt.bfloat16
i16 = mybir.dt.int16
Alu = mybir.AluOpType
P = 128
BF16 = ml_dtypes.bfloat16


def build_dcn(C=256, Cout=256, H=64, W=64, KH=3, KW=3, CHUNK_JT=8):
    HW = H * W
    S = HW // P              # 32 pixel slots of 128
    NT = KH * KW             # 9 taps
    CB = C // P              # 2 input-channel blocks
    MB = Cout // P           # 2 output-channel blocks
    assert S % CHUNK_JT == 0
    n_chunks = S // CHUNK_JT
    JC = CHUNK_JT * P        # 1024 pixels per chunk
    SWC = JC // 16           # idx columns per chunk (16-wrap layout)
    NNB = JC // 512          # moving-dim blocks for the GEMM
    TROWS = (H + 1) * W      # pair-table rows

    nc = bacc.Bacc("TRN2", target_bir_lowering=False, debug=False)

    # one extra zero row backs the overlapping pair view's last entry
    tbl = nc.declare_dram_parameter("tbl", [TROWS + 1, 2 * C], bf16,
                                    isOutput=False)
    idx = nc.declare_dram_parameter("idx", [P, NT, n_chunks * SWC], i16,
                                    isOutput=False)
    w4 = nc.declare_dram_parameter("w4", [P, NT, S, 4], bf16, isOutput=False)
    wt = nc.declare_dram_parameter("wt", [P, NT, CB, Cout], bf16, isOutput=False)
    out = nc.declare_dram_parameter("out", [Cout, HW], bf16, isOutput=True)

    with tile.TileContext(nc) as tc:
        with tc.tile_pool(name="persist", bufs=1) as pp:
            wtb = pp.tile([P, NT, CB, Cout], bf16, name="wtb")
            w4b = pp.tile([P, NT, S, 4], bf16, name="w4b")
            w4t = pp.tile([P, NT, S, 4], f32, name="w4t")
            idxt = pp.tile([P, NT, n_chunks * SWC], i16, name="idxt")
            ident = pp.tile([P, P], bf16, name="ident")

            nc.sync.dma_start(out=idxt[:], in_=idx[:])
            nc.sync.dma_start(out=w4b[:], in_=w4[:])
            nc.sync.dma_start(out=wtb[:], in_=wt[:])
            # upconvert weights on DVE (idle at startup) - halves the w4 DMA
            nc.vector.tensor_copy(out=w4t[:], in_=w4b[:])
            make_identity(nc, ident[:])

            # overlapping-pair view: entry i covers table elements
            # [2C*i, 2C*i + 4C) -> one descriptor = 4 bilinear neighbors.
            tbl_pairs = bass.AP(tbl[:].tensor, 0, [[2 * C, TROWS], [1, 4 * C]])

            HJT = CHUNK_JT // 2          # jts per output-column half
            with (
                tc.tile_pool(name="gather", bufs=4) as g_pool,
                tc.tile_pool(name="prod", bufs=24) as pr_pool,
                tc.tile_pool(name="vout", bufs=8) as v_pool,
                tc.tile_pool(name="obuf", bufs=2) as o_pool,
                tc.tile_pool(name="psum_out", bufs=1, space="PSUM") as pso_pool,
                tc.tile_pool(name="psum_val", bufs=4, space="PSUM") as psv_pool,
            ):
                for ch in range(n_chunks):
                    out_ps = [
                        pso_pool.tile([P, JC], f32, space="PSUM", name=f"out_ps{m}")
                        for m in range(MB)
                    ]
                    for k in range(NT):
                        g = g_pool.tile([P, CHUNK_JT, 4 * C], bf16, name="g")
                        nc.gpsimd.dma_gather(
                            g[:], tbl_pairs,
                            idxt[:, k, ch * SWC:(ch + 1) * SWC],
                            JC, JC, 4 * C, elem_step=2 * C,
                        )
                        # per-jt pr tiles: PE transposes stream right behind
                        # the DVE multiplies instead of waiting for all 32
                        prs = []
                        for jt in range(CHUNK_JT):
                            s_idx = ch * CHUNK_JT + jt
                            pr = pr_pool.tile([P, 4, C], bf16, name="pr")
                            prs.append(pr)
                            for n in range(4):
                                nc.vector.tensor_scalar(
                                    out=pr[:, n, :],
                                    in0=g[:, jt, n * C:(n + 1) * C],
                                    scalar1=w4t[:, k, s_idx, n:n + 1],
                                    scalar2=None, op0=Alu.mult,
                                )
                        # val/vsb split per (cb, column-half): Act half-copies
                        # (570ns) start mid-iteration, so no GEMM ever waits
                        # on a copy still in flight
                        vhalf = {}
                        for cb in range(CB):
                            for h in range(2):
                                val_ps = psv_pool.tile([P, HJT * P], f32,
                                                       space="PSUM",
                                                       name="val_ps")
                                for j2 in range(HJT):
                                    jt = h * HJT + j2
                                    for n in range(4):
                                        nc.tensor.matmul(
                                            out=val_ps[:, j2 * P:(j2 + 1) * P],
                                            lhsT=prs[jt][:, n,
                                                         cb * P:(cb + 1) * P],
                                            rhs=ident[:],
                                            start=(n == 0), stop=(n == 3),
                                        )
                                vs = v_pool.tile([P, HJT * P], bf16, name="vs")
                                nc.scalar.copy(out=vs[:], in_=val_ps[:])
                                vhalf[(cb, h)] = vs
                        for h in range(2):
                            for mb in range(MB):
                                for cb in range(CB):
                                    nsl = slice(h * 512, (h + 1) * 512)
                                    nc.tensor.matmul(
                                        out=out_ps[mb][:, nsl],
                                        lhsT=wtb[:, k, cb, mb * P:(mb + 1) * P],
                                        rhs=vhalf[(cb, h)][:],
                                        start=(k == 0 and cb == 0),
                                        stop=(k == NT - 1 and cb == CB - 1),
                                    )
                    for mb in range(MB):
                        ob = o_pool.tile([P, JC], bf16, name="ob")
                        nc.scalar.copy(out=ob[:], in_=out_ps[mb][:])
                        nc.sync.dma_start(
                            out=out[mb * P:(mb + 1) * P, ch * JC:(ch + 1) * JC],
                            in_=ob[:],
                        )

    nc.compile()
    return nc


def host_prep(x_b, offset_b, weight, H, W, KH, KW, PAD):
    """Per-core input map from one batch slice (numpy, f32)."""
    C = x_b.shape[0]
    Cout = weight.shape[0]
    HW = H * W
    S = HW // P
    NT = KH * KW
    CB = C // P

    # pair-row gather table: entry r=(y0+1)*W+x holds rows (y0, y0+1) at col x
    xt = x_b.reshape(C, H, W).transpose(1, 2, 0).astype(np.float32)  # [H, W, C]
    Z = np.zeros((H + 2, W, C), np.float32)
    Z[1:H + 1] = xt
    T = np.concatenate([Z[0:H + 1], Z[1:H + 2]], axis=-1)  # [(H+1), W, 2C]
    tbl = np.zeros(((H + 1) * W + 1, 2 * C), np.float32)
    tbl[:-1] = T.reshape((H + 1) * W, 2 * C)
    tbl = tbl.astype(BF16)

    # sample coords per (tap, pixel)
    off = offset_b.reshape(NT, 2, HW).astype(np.float32)
    j = np.arange(HW)
    ks = np.arange(NT)
    by = (j[None, :] // W - PAD + (ks // KW)[:, None]).astype(np.float32)
    bx = (j[None, :] % W - PAD + (ks % KW)[:, None]).astype(np.float32)
    py = by + off[:, 0]
    px = bx + off[:, 1]
    y0 = np.floor(py)
    x0 = np.floor(px)
    ly = (py - y0).astype(np.float32)
    lx = (px - x0).astype(np.float32)
    qy = np.clip(y0, -1, H - 1)
    sx = np.clip(x0, 0, W - 2)
    idx_lin = ((qy + 1) * W + sx).astype(np.int16)  # [NT, HW]

    wy0 = (1.0 - ly) * ((y0 >= 0) & (y0 <= H - 1))
    wyB = ly * ((y0 >= -1) & (y0 <= H - 2))
    vx0 = (x0 >= 0) & (x0 <= W - 1)
    vx1 = (x0 >= -1) & (x0 <= W - 2)
    wxA = (1.0 - lx) * vx0 * (x0 == sx) + lx * vx1 * ((x0 + 1) == sx)
    wxB = (1.0 - lx) * vx0 * (x0 == (sx + 1)) + lx * vx1 * ((x0 + 1) == (sx + 1))
    # piece order matches the gathered 4C row: [y0|x0, y1|x0, y0|x1, y1|x1]
    w4 = np.stack([wy0 * wxA, wyB * wxA, wy0 * wxB, wyB * wxB],
                  axis=-1).astype(np.float32)  # [NT, HW, 4]
    w4d = np.ascontiguousarray(
        w4.reshape(NT, S, P, 4).transpose(2, 0, 1, 3))  # [P, NT, S, 4]

    # 16-wrap idx layout: slice column c of chunk ch, partition q -> sample
    # i = c*16 + q (i = chunk-local pixel), replicated over 8 partition groups
    idxw = idx_lin.reshape(NT, HW // 16, 16).transpose(2, 0, 1)  # [16, NT, HW/16]
    idxw = np.ascontiguousarray(np.tile(idxw, (8, 1, 1))).astype(np.int16)

    wtd = np.ascontiguousarray(
        weight.reshape(Cout, CB, P, NT).transpose(2, 3, 1, 0)).astype(BF16)
    return {"tbl": tbl, "idx": idxw, "w4": w4d, "wt": wtd}


_NC_CACHE = {}


def _get_nc(key, **kw):
    if key not in _NC_CACHE:
        _NC_CACHE[key] = build_dcn(**kw)
    return _NC_CACHE[key]


def kernel(x, offset, weight):
    x = np.asarray(x, dtype=np.float32)
    offset = np.asarray(offset, dtype=np.float32)
    weight = np.asarray(weight, dtype=np.float32)
    B, C, H, W = x.shape
    Cout = weight.shape[0]
    KH, KW = weight.shape[2], weight.shape[3]
    assert B == 8 and C % 128 == 0 and Cout % 128 == 0
    nc = _get_nc((C, Cout, H, W, KH, KW), C=C, Cout=Cout, H=H, W=W,
                 KH=KH, KW=KW, CHUNK_JT=8)
    in_maps = [host_prep(x[b], offset[b], weight, H, W, KH, KW, 1)
               for b in range(B)]
    res = run_bass_kernel_spmd(nc, in_maps, list(range(B)))
    out = np.stack([
        np.asarray(res.results[b]["out"]).astype(np.float32).reshape(Cout, H, W)
        for b in range(B)
    ])
    return out


# revision 23
# speedup vs baseline: 1.0036x; 1.0036x over previous
"""Self-contained Trainium2 (Bass/Tile) DeformConv2d kernel.

kernel(x, offset, weight) -> np.ndarray [B, Cout, H, W] float32.
Data-parallel over batch: one SPMD Bass program per NeuronCore (8 cores).

v2 design (vs v1 baseline):
- All bilinear weights / gather indices / gather table are built on the host
  (numpy) and passed as DRAM inputs; no on-device prep phases.
- Gather table is a bf16 "pair-row" layout: entry (y, x) holds image rows y
  and y+1 at column x concatenated (2C values). One 4C-value descriptor per
  sample covers all 4 bilinear neighbors (half the gather instructions and
  descriptor-gen of v1; DMA bytes unchanged).
- DVE does only the 4 per-sample weight multiplies (tensor_scalar, 4x mode).
- PE transpose-accumulates the 4 pieces into channel-major val (PSUM), then
  runs the per-tap GEMM.
- All PSUM->SBUF copies run on the otherwise-idle Activation engine.
- Output is written bf16 and upcast on the host.
"""
import sys
import numpy as np
import ml_dtypes

for _p in ("/opt/trn_rl_repo",):
    if _p not in sys.path:
        sys.path.insert(0, _p)

import concourse.bass as bass
import concourse.mybir as mybir
import concourse.tile as tile
from concourse import bacc
from concourse.masks import make_identity
from concourse.bass_utils import run_bass_kernel_spmd

f32 = mybir.dt.float32
bf16 = mybir.dt.bfloat16
i16 = mybir.dt.int16
Alu = mybir.AluOpType
P = 128
BF16 = ml_dtypes.bfloat16


def build_dcn(C=256, Cout=256, H=64, W=64, KH=3, KW=3, CHUNK_JT=8):
    HW = H * W
    S = HW // P              # 32 pixel slots of 128
    NT = KH * KW             # 9 taps
    CB = C // P              # 2 input-channel blocks
    MB = Cout // P           # 2 output-channel blocks
    assert S % CHUNK_JT == 0
    n_chunks = S // CHUNK_JT
    JC = CHUNK_JT * P        # 1024 pixels per chunk
    SWC = JC // 16           # idx columns per chunk (16-wrap layout)
    NNB = JC // 512          # moving-dim blocks for the GEMM
    TROWS = (H + 1) * W      # pair-table rows

    nc = bacc.Bacc("TRN2", target_bir_lowering=False, debug=False)

    # one extra zero row backs the overlapping pair view's last entry
    tbl = nc.declare_dram_parameter("tbl", [TROWS + 1, 2 * C], bf16,
                                    isOutput=False)
    idx = nc.declare_dram_parameter("idx", [P, NT, n_chunks * SWC], i16,
                                    isOutput=False)
    w4 = nc.declare_dram_parameter("w4", [P, NT, S, 4], bf16, isOutput=False)
    wt = nc.declare_dram_parameter("wt", [P, NT, CB, Cout], bf16, isOutput=False)
    out = nc.declare_dram_parameter("out", [Cout, HW], bf16, isOutput=True)

    with tile.TileContext(nc) as tc:
        with tc.tile_pool(name="persist", bufs=1) as pp:
            wtb = pp.tile([P, NT, CB, Cout], bf16, name="wtb")
            w4b = pp.tile([P, NT, S, 4], bf16, name="w4b")
            w4t = pp.tile([P, NT, S, 4], f32, name="w4t")
            idxt = pp.tile([P, NT, n_chunks * SWC], i16, name="idxt")
            ident = pp.tile([P, P], bf16, name="ident")

            nc.sync.dma_start(out=idxt[:], in_=idx[:])
            nc.sync.dma_start(out=w4b[:], in_=w4[:])
            nc.sync.dma_start(out=wtb[:], in_=wt[:])
            # upconvert weights on DVE (idle at startup) - halves the w4 DMA
            nc.vector.tensor_copy(out=w4t[:], in_=w4b[:])
            make_identity(nc, ident[:])

            # overlapping-pair view: entry i covers table elements
            # [2C*i, 2C*i + 4C) -> one descriptor = 4 bilinear neighbors.
            tbl_pairs = bass.AP(tbl[:].tensor, 0, [[2 * C, TROWS], [1, 4 * C]])

            HJT = CHUNK_JT // 2          # jts per output-column half
            with (
                tc.tile_pool(name="gather", bufs=4) as g_pool,
                tc.tile_pool(name="prod", bufs=24) as pr_pool,
                tc.tile_pool(name="vout", bufs=8) as v_pool,
                tc.tile_pool(name="obuf", bufs=2) as o_pool,
                tc.tile_pool(name="psum_out", bufs=1, space="PSUM") as pso_pool,
                tc.tile_pool(name="psum_val", bufs=4, space="PSUM") as psv_pool,
            ):
                def do_block(out_ps, ch, k, h_list):
                    """One tap's gather+compute for the given column halves."""
                    jts = [h * HJT + j for h in h_list for j in range(HJT)]
                    nj = len(jts)
                    ni = nj * P
                    g = g_pool.tile([P, nj, 4 * C], bf16, name="g")
                    col0 = ch * SWC + (jts[0] * P) // 16
                    nc.gpsimd.dma_gather(
                        g[:], tbl_pairs,
                        idxt[:, k, col0:col0 + ni // 16],
                        ni, ni, 4 * C, elem_step=2 * C,
                    )
                    # per-jt pr tiles: PE transposes stream right behind
                    # the DVE multiplies instead of waiting for all of them
                    prs = {}
                    for i, jt in enumerate(jts):
                        s_idx = ch * CHUNK_JT + jt
                        pr = pr_pool.tile([P, 4, C], bf16, name="pr")
                        prs[jt] = pr
                        for n in range(4):
                            nc.vector.tensor_scalar(
                                out=pr[:, n, :],
                                in0=g[:, i, n * C:(n + 1) * C],
                                scalar1=w4t[:, k, s_idx, n:n + 1],
                                scalar2=None, op0=Alu.mult,
                            )
                    # val/vsb split per (cb, column-half): Act half-copies
                    # (570ns) start mid-iteration, so no GEMM ever waits
                    # on a copy still in flight
                    vhalf = {}
                    for cb in range(CB):
                        for h in h_list:
                            val_ps = psv_pool.tile([P, HJT * P], f32,
                                                   space="PSUM", name="val_ps")
                            for j2 in range(HJT):
                                jt = h * HJT + j2
                                for n in range(4):
                                    nc.tensor.matmul(
                                        out=val_ps[:, j2 * P:(j2 + 1) * P],
                                        lhsT=prs[jt][:, n, cb * P:(cb + 1) * P],
                                        rhs=ident[:],
                                        start=(n == 0), stop=(n == 3),
                                    )
                            vs = v_pool.tile([P, HJT * P], bf16, name="vs")
                            nc.scalar.copy(out=vs[:], in_=val_ps[:])
                            vhalf[(cb, h)] = vs
                    for h in h_list:
                        for mb in range(MB):
                            for cb in range(CB):
                                nsl = slice(h * 512, (h + 1) * 512)
                                nc.tensor.matmul(
                                    out=out_ps[mb][:, nsl],
                                    lhsT=wtb[:, k, cb, mb * P:(mb + 1) * P],
                                    rhs=vhalf[(cb, h)][:],
                                    start=(k == 0 and cb == 0),
                                    stop=(k == NT - 1 and cb == CB - 1),
                                )

                for ch in range(n_chunks):
                    out_ps = [
                        pso_pool.tile([P, JC], f32, space="PSUM", name=f"out_ps{m}")
                        for m in range(MB)
                    ]
                    for k in range(NT):
                        if ch == n_chunks - 1 and k == NT - 1:
                            # split the final tap into two half-gathers so the
                            # post-gather drain pipeline is half as deep
                            do_block(out_ps, ch, k, [0])
                            do_block(out_ps, ch, k, [1])
                        else:
                            do_block(out_ps, ch, k, [0, 1])
                    for mb in range(MB):
                        ob = o_pool.tile([P, JC], bf16, name="ob")
                        nc.scalar.copy(out=ob[:], in_=out_ps[mb][:])
                        nc.sync.dma_start(
                            out=out[mb * P:(mb + 1) * P, ch * JC:(ch + 1) * JC],
                            in_=ob[:],
                        )

    nc.compile()
    return nc


def host_prep(x_b, offset_b, weight, H, W, KH, KW, PAD):
    """Per-core input map from one batch slice (numpy, f32)."""
    C = x_b.shape[0]
    Cout = weight.shape[0]
    HW = H * W
    S = HW // P
    NT = KH * KW
    CB = C // P

    # pair-row gather table: entry r=(y0+1)*W+x holds rows (y0, y0+1) at col x
    xt = x_b.reshape(C, H, W).transpose(1, 2, 0).astype(np.float32)  # [H, W, C]
    Z = np.zeros((H + 2, W, C), np.float32)
    Z[1:H + 1] = xt
    T = np.concatenate([Z[0:H + 1], Z[1:H + 2]], axis=-1)  # [(H+1), W, 2C]
    tbl = np.zeros(((H + 1) * W + 1, 2 * C), np.float32)
    tbl[:-1] = T.reshape((H + 1) * W, 2 * C)
    tbl = tbl.astype(BF16)

    # sample coords per (tap, pixel)
    off = offset_b.reshape(NT, 2, HW).astype(np.float32)
    j = np.arange(HW)
    ks = np.arange(NT)
    by = (j[None, :] // W - PAD + (ks // KW)[:, None]).astype(np.float32)
    bx = (j[None, :] % W - PAD + (ks % KW)[:, None]).astype(np.float32)
    py = by + off[:, 0]
    px = bx + off[:, 1]
    y0 = np.floor(py)
    x0 = np.floor(px)
    ly = (py - y0).astype(np.float32)
    lx = (px - x0).astype(np.float32)
    qy = np.clip(y0, -1, H - 1)
    sx = np.clip(x0, 0, W - 2)
    idx_lin = ((qy + 1) * W + sx).astype(np.int16)  # [NT, HW]

    wy0 = (1.0 - ly) * ((y0 >= 0) & (y0 <= H - 1))
    wyB = ly * ((y0 >= -1) & (y0 <= H - 2))
    vx0 = (x0 >= 0) & (x0 <= W - 1)
    vx1 = (x0 >= -1) & (x0 <= W - 2)
    wxA = (1.0 - lx) * vx0 * (x0 == sx) + lx * vx1 * ((x0 + 1) == sx)
    wxB = (1.0 - lx) * vx0 * (x0 == (sx + 1)) + lx * vx1 * ((x0 + 1) == (sx + 1))
    # piece order matches the gathered 4C row: [y0|x0, y1|x0, y0|x1, y1|x1]
    w4 = np.stack([wy0 * wxA, wyB * wxA, wy0 * wxB, wyB * wxB],
                  axis=-1).astype(np.float32)  # [NT, HW, 4]
    w4d = np.ascontiguousarray(
        w4.reshape(NT, S, P, 4).transpose(2, 0, 1, 3)).astype(BF16)  # [P,NT,S,4]

    # 16-wrap idx layout: slice column c of chunk ch, partition q -> sample
    # i = c*16 + q (i = chunk-local pixel), replicated over 8 partition groups
    idxw = idx_lin.reshape(NT, HW // 16, 16).transpose(2, 0, 1)  # [16, NT, HW/16]
    idxw = np.ascontiguousarray(np.tile(idxw, (8, 1, 1))).astype(np.int16)

    wtd = np.ascontiguousarray(
        weight.reshape(Cout, CB, P, NT).transpose(2, 3, 1, 0)).astype(BF16)
    return {"tbl": tbl, "idx": idxw, "w4": w4d, "wt": wtd}


_NC_CACHE = {}


def _get_nc(key, **kw):
    if key not in _NC_CACHE:
        _NC_CACHE[key] = build_dcn(**kw)
    return _NC_CACHE[key]


def kernel(x, offset, weight):
    x = np.asarray(x, dtype=np.float32)
    offset = np.asarray(offset, dtype=np.float32)
    weight = np.asarray(weight, dtype=np.float32)
    B, C, H, W = x.shape
    Cout = weight.shape[0]
    KH, KW = weight.shape[2], weight.shape[3]
    assert B == 8 and C % 128 == 0 and Cout % 128 == 0
    nc = _get_nc((C, Cout, H, W, KH, KW), C=C, Cout=Cout, H=H, W=W,
                 KH=KH, KW=KW, CHUNK_JT=8)
    in_maps = [host_prep(x[b], offset[b], weight, H, W, KH, KW, 1)
               for b in range(B)]
    res = run_bass_kernel_spmd(nc, in_maps, list(range(B)))
    out = np.stack([
        np.asarray(res.results[b]["out"]).astype(np.float32).reshape(Cout, H, W)
        for b in range(B)
    ])
    return out
